# revision 1
# baseline (speedup 1.0000x reference)
"""Trainium2 Bass kernel for CategoricalDnn: embedding gather + BatchNorm(train) + ReLU + concat.

Reference computation (B=65536, F=32, V=1001, D=64, N_NUM=16):
    emb[b,f,:]  = tables[f, cat_idx[b,f], :]
    mean/var    = biased batch stats of emb over b
    normed      = (emb - mean) * rsqrt(var+eps) * gamma + beta
    out         = concat([relu(normed).reshape(B, F*D), numerical], axis=1)

Strategy (8 NeuronCores, data-parallel over the batch):
  * Host: linearize cat_idx into int32 row indices of the flattened
    [F*V, 64] table, laid out [128, NT*F] so tile t / feature f offsets sit
    at column t*F+f (one offset per SBUF partition = one gathered row).
  * Phase 1 (device): each core gathers its 8192x32 embedding rows tile by
    tile ([128, 2048] per tile) with per-feature indirect DMAs (the
    compiler-supported DynamicAP gather: 128 rows x 256B per instruction),
    accumulating sum on DVE and sum-of-squares via ACT square + DVE add,
    while spilling each raw tile to a DRAM scratch over HWDGE; one PE matmul
    against a ones vector reduces the accumulators across partitions.
  * AllReduce (device collective) of the [1, 4096] partial stats across the 8
    cores; each core then computes scale = gamma*rsqrt(var+eps) and
    shift = beta - mean*scale and broadcasts them to 128 partitions via PE.
  * Phase 2 (device, pure HWDGE streaming - no gpsimd): reload each scratch
    tile, x*scale+shift on DVE, ReLU on ACT, splice the numerical columns,
    store [128, 2064] output blocks.
"""

import sys

import numpy as np

if "/opt/trn_rl_repo" not in sys.path:
    sys.path.insert(0, "/opt/trn_rl_repo")

import concourse.bacc as bacc
import concourse.bass as bass
import concourse.mybir as mybir
from concourse.bass_utils import run_bass_kernel_spmd

# Problem constants (hardcoded per harness contract).
B, F, V, D, N_NUM = 65536, 32, 1001, 64, 16
EPS = 1e-5
NCORES = 8
BC = B // NCORES          # 8192 batch rows per core
TILE = 128                # batch rows per gather tile
NT = BC // TILE           # 64 tiles per core
FD = F * D                # 2048
OW = FD + N_NUM           # 2064 output columns
R = F * V                 # 32032 flat table rows
P = 128

f32 = mybir.dt.float32
i32 = mybir.dt.int32


def _build_nc() -> bass.Bass:
    nc = bacc.Bacc("TRN2", target_bir_lowering=False, debug=False,
                   num_devices=NCORES)

    tab = nc.dram_tensor("tab", [R, D], f32, kind="ExternalInput")
    idx = nc.dram_tensor("idx", [P, NT * F], i32, kind="ExternalInput")
    nume = nc.dram_tensor("nume", [P, NT * N_NUM], f32, kind="ExternalInput")
    gb = nc.dram_tensor("gb", [2, FD], f32, kind="ExternalInput")
    out = nc.dram_tensor("out", [BC, OW], f32, kind="ExternalOutput")
    scratch = nc.dram_tensor("scratch", [BC, FD], f32, kind="Internal")
    cc_in = nc.dram_tensor("cc_in", [1, 2 * FD], f32, kind="Internal")
    cc_out = nc.dram_tensor("cc_out", [1, 2 * FD], f32, kind="Internal",
                            addr_space="Shared")

    from contextlib import ExitStack
    with ExitStack() as ctx:
        sb = lambda name, shape, dt: ctx.enter_context(
            nc.sbuf_tensor(name, shape, dt))
        idx_sb = sb("idx_sb", [P, NT * F], i32)
        gbufs = [sb(f"gb{k}", [P, OW], f32) for k in range(6)]
        q0 = sb("q0", [P, FD], f32)
        q1 = sb("q1", [P, FD], f32)
        acc_s = sb("acc_s", [P, FD], f32)
        acc_q = sb("acc_q", [P, FD], f32)
        sc_bc = sb("sc_bc", [P, FD], f32)
        sh_bc = sb("sh_bc", [P, FD], f32)
        num_sb = sb("num_sb", [P, NT * N_NUM], f32)
        ga_sb = sb("ga_sb", [1, FD], f32)
        be_sb = sb("be_sb", [1, FD], f32)
        stat_s = sb("stat_s", [1, FD], f32)
        stat_q = sb("stat_q", [1, FD], f32)
        statg = sb("statg", [1, 2 * FD], f32)
        mrow = sb("mrow", [1, FD], f32)
        vrow = sb("vrow", [1, FD], f32)
        srow = sb("srow", [1, FD], f32)
        trow = sb("trow", [1, FD], f32)
        ones_c = sb("ones_c", [P, 1], f32)
        ones_r = sb("ones_r", [1, P], f32)
        eps_row = sb("eps_row", [1, 1], f32)

        ps_stat = ctx.enter_context(nc.psum_tensor("ps_stat", [1, FD], f32))
        ps_bc = ctx.enter_context(nc.psum_tensor("ps_bc", [P, FD], f32))

        sem = lambda name: ctx.enter_context(nc.semaphore(name))
        s_ld = sem("s_ld")
        s_g1 = [sem(f"s_g1_{k}") for k in range(6)]
        s_a1 = sem("s_a1")
        s_v1 = sem("s_v1")
        s_pe = sem("s_pe")
        s_ax = sem("s_ax")
        s_cc = sem("s_cc")
        s_ccd = sem("s_ccd")
        s_cc2 = sem("s_cc2")
        s_g2 = [sem(f"s_g2_{k}") for k in range(6)]
        s_v2 = sem("s_v2")
        s_r2 = sem("s_r2")
        s_w2 = [sem(f"s_w2_{k}") for k in range(6)]
        s_sc = [sem(f"s_sc_{k}") for k in range(6)]
        s_vi = sem("s_vi")
        s_vm = sem("s_vm")
        s_vs = sem("s_vs")

        gbuf = gbufs
        qbuf = [q0, q1]
        NB = 6
        # tiles mapped to buffer k (NT need not divide NB)
        cnt = lambda k: (NT - k + NB - 1) // NB

        with nc.Block("main") as block:

            @block.sync
            def _(sync):
                sync.dma_start(idx_sb[:, :], idx[:, :]).then_inc(s_ld, 16)
                sync.dma_start(num_sb[:, :], nume[:, :]).then_inc(s_ld, 16)
                sync.dma_start(ga_sb[:, :], gb[0:1, :]).then_inc(s_ld, 16)
                sync.dma_start(be_sb[:, :], gb[1:2, :]).then_inc(s_ld, 16)
                # phase-1 scratch spills
                for t in range(NT):
                    sync.wait_ge(s_g1[t % NB], 512 * (t // NB + 1))
                    sync.dma_start(
                        scratch[t * TILE:(t + 1) * TILE, :],
                        gbuf[t % NB][:, :FD],
                    ).then_inc(s_sc[t % NB], 16)
                # phase-2 output stores (reloads run on gpsimd)
                for t in range(NT):
                    sync.wait_ge(s_r2, t + 1)
                    sync.dma_start(
                        out[t * TILE:(t + 1) * TILE, :], gbuf[t % NB][:, :],
                    ).then_inc(s_w2[t % NB], 16)
                for k in range(NB):
                    sync.wait_ge(s_w2[k], 16 * cnt(k))

            @block.gpsimd
            def _(gpsimd):
                gpsimd.wait_ge(s_ld, 64)  # all initial loads done
                # phase 1 gathers: 32 per-feature indirect DMAs per tile
                for t in range(NT):
                    if t >= NB:
                        gpsimd.wait_ge(s_v1, t - NB + 1)
                        gpsimd.wait_ge(s_a1, t - NB + 1)
                        gpsimd.wait_ge(s_sc[t % NB], 16 * (t // NB))
                    for f in range(F):
                        gpsimd.indirect_dma_start(
                            out=gbuf[t % NB][:, f * D:(f + 1) * D],
                            out_offset=None,
                            in_=tab[:, :],
                            in_offset=bass.IndirectOffsetOnAxis(
                                ap=idx_sb[:, t * F + f:t * F + f + 1], axis=0),
                        ).then_inc(s_g1[t % NB], 16)
                # prologue phase-2 reloads (hide latency under the collective)
                for t in range(min(NB, NT)):
                    gpsimd.wait_ge(s_v1, NT)
                    gpsimd.wait_ge(s_a1, NT)
                    gpsimd.wait_ge(s_sc[t % NB], 16 * cnt(t % NB))
                    gpsimd.dma_start(
                        gbuf[t % NB][:, :FD],
                        scratch[t * TILE:(t + 1) * TILE, :],
                    ).then_inc(s_g2[t % NB], 16)
                # stats allreduce
                gpsimd.wait_ge(s_vs, 2)   # stat rows written
                gpsimd.dma_start(cc_in[:, :FD], stat_s[:, :]).then_inc(s_cc, 16)
                gpsimd.dma_start(cc_in[:, FD:], stat_q[:, :]).then_inc(s_cc, 16)
                gpsimd.wait_ge(s_cc, 32)
                gpsimd.collective_compute(
                    "AllReduce",
                    mybir.AluOpType.add,
                    replica_groups=[list(range(NCORES))],
                    ins=[cc_in.ap().opt()],
                    outs=[cc_out.ap().opt()],
                ).then_inc(s_ccd, 1)
                gpsimd.wait_ge(s_ccd, 1)
                gpsimd.dma_start(statg[:, :], cc_out[:, :]).then_inc(s_cc2, 16)
                # remaining phase-2 reloads, prefetched NB tiles ahead
                for t in range(min(NB, NT), NT):
                    gpsimd.wait_ge(s_w2[t % NB], 16 * (t // NB))
                    gpsimd.dma_start(
                        gbuf[t % NB][:, :FD],
                        scratch[t * TILE:(t + 1) * TILE, :],
                    ).then_inc(s_g2[t % NB], 16)

            @block.vector
            def _(vector):
                # Same-engine RAW/WAW chains need explicit sems (deep DVE
                # pipeline): s_vi orders the boot memsets, s_v1/s_v2 the
                # per-tile groups, s_vs chains the stats section.
                vector.memset(acc_s[:, :], 0.0)
                vector.memset(acc_q[:, :], 0.0)
                vector.memset(ones_c[:, :], 1.0)
                vector.memset(ones_r[:, :], 1.0)
                vector.memset(eps_row[:, :], float(EPS)).then_inc(s_vi, 1)
                # phase 1 accumulate
                for t in range(NT):
                    if t == 0:
                        vector.wait_ge(s_vi, 1)
                    else:
                        vector.wait_ge(s_v1, t)
                    vector.wait_ge(s_g1[t % NB], 512 * (t // NB + 1))
                    vector.tensor_add(acc_s[:, :], acc_s[:, :],
                                      gbuf[t % NB][:, :FD])
                    vector.wait_ge(s_a1, t + 1)
                    vector.tensor_add(
                        acc_q[:, :], acc_q[:, :], qbuf[t % 2][:, :],
                    ).then_inc(s_v1, 1)
                # ---- stats section: every DVE op self-chained via s_vs ----
                vsn = [0]

                def vstep(emit, *waits):
                    vector.wait_ge(s_vs, vsn[0])
                    for w_sem, w_val in waits:
                        vector.wait_ge(w_sem, w_val)
                    inst = emit()
                    vsn[0] += 1
                    inst.then_inc(s_vs, 1)
                    return inst

                vstep(lambda: vector.tensor_copy(stat_s[:, :], ps_stat[:, :]),
                      (s_v1, NT), (s_pe, 1))                       # s_vs=1
                vstep(lambda: vector.tensor_copy(stat_q[:, :], ps_stat[:, :]),
                      (s_pe, 2))                                   # s_vs=2
                vstep(lambda: vector.tensor_scalar_mul(
                    mrow[:, :], statg[:, :FD], 1.0 / B),
                    (s_ld, 64), (s_cc2, 16))                       # s_vs=3
                vstep(lambda: vector.tensor_scalar_mul(
                    vrow[:, :], statg[:, FD:], 1.0 / B))           # s_vs=4
                vstep(lambda: vector.tensor_mul(
                    srow[:, :], mrow[:, :], mrow[:, :]))           # s_vs=5
                vstep(lambda: vector.tensor_sub(
                    vrow[:, :], vrow[:, :], srow[:, :]))           # var, s_vs=6
                vstep(lambda: vector.reciprocal(vrow[:, :], vrow[:, :]),
                      (s_ax, 1))                                   # s_vs=7
                vstep(lambda: vector.tensor_mul(
                    srow[:, :], ga_sb[:, :], vrow[:, :]))          # s_vs=8
                vstep(lambda: vector.tensor_mul(
                    mrow[:, :], mrow[:, :], srow[:, :]))           # s_vs=9
                vstep(lambda: vector.tensor_sub(
                    trow[:, :], be_sb[:, :], mrow[:, :]))          # shift, s_vs=10
                vstep(lambda: vector.tensor_copy(sc_bc[:, :], ps_bc[:, :]),
                      (s_pe, 3))                                   # s_vs=11
                vstep(lambda: vector.tensor_copy(sh_bc[:, :], ps_bc[:, :]),
                      (s_pe, 4))                                   # s_vs=12
                n_vs = vsn[0]
                # phase 2 normalize
                for t in range(NT):
                    if t == 0:
                        vector.wait_ge(s_vs, n_vs)
                    else:
                        vector.wait_ge(s_v2, t)
                    vector.wait_ge(s_g2[t % NB], 16 * (t // NB + 1))
                    gt = gbuf[t % NB]
                    vector.tensor_mul(
                        gt[:, :FD], gt[:, :FD], sc_bc[:, :]).then_inc(s_vm, 1)
                    vector.wait_ge(s_vm, t + 1)
                    vector.tensor_add(gt[:, :FD], gt[:, :FD], sh_bc[:, :])
                    vector.tensor_copy(
                        gt[:, FD:],
                        num_sb[:, t * N_NUM:(t + 1) * N_NUM],
                    ).then_inc(s_v2, 1)

            @block.scalar
            def _(scalar):
                # phase 1 squares
                for t in range(NT):
                    scalar.wait_ge(s_g1[t % NB], 512 * (t // NB + 1))
                    if t >= 2:
                        scalar.wait_ge(s_v1, t - 1)
                    scalar.square(
                        qbuf[t % 2][:, :], gbuf[t % NB][:, :FD],
                    ).then_inc(s_a1, 1)
                # sqrt(var + eps)
                scalar.wait_ge(s_vs, 6)
                scalar.activation(
                    vrow[:, :], vrow[:, :], mybir.ActivationFunctionType.Sqrt,
                    bias=eps_row[:, :],
                ).then_inc(s_ax, 1)
                # phase 2 relu
                for t in range(NT):
                    scalar.wait_ge(s_v2, t + 1)
                    gt = gbuf[t % NB]
                    scalar.activation(
                        gt[:, :FD], gt[:, :FD],
                        mybir.ActivationFunctionType.Relu,
                    ).then_inc(s_r2, 1)

            @block.tensor
            def _(tensor):
                NB = FD // 512  # 4 psum banks
                # colsum of acc_s, then acc_q (ps_stat reused in between)
                tensor.wait_ge(s_v1, NT)
                for k in range(NB):
                    mm = tensor.matmul(
                        ps_stat[:, k * 512:(k + 1) * 512], ones_c[:, :],
                        acc_s[:, k * 512:(k + 1) * 512], start=True, stop=True)
                mm.then_inc(s_pe, 1)
                tensor.wait_ge(s_vs, 1)
                for k in range(NB):
                    mm = tensor.matmul(
                        ps_stat[:, k * 512:(k + 1) * 512], ones_c[:, :],
                        acc_q[:, k * 512:(k + 1) * 512], start=True, stop=True)
                mm.then_inc(s_pe, 1)
                # broadcast scale and shift across partitions (ps_bc reused)
                tensor.wait_ge(s_vs, 10)
                for k in range(NB):
                    mm = tensor.matmul(
                        ps_bc[:, k * 512:(k + 1) * 512], ones_r[:, :],
                        srow[:, k * 512:(k + 1) * 512], start=True, stop=True)
                mm.then_inc(s_pe, 1)
                tensor.wait_ge(s_vs, 11)
                for k in range(NB):
                    mm = tensor.matmul(
                        ps_bc[:, k * 512:(k + 1) * 512], ones_r[:, :],
                        trow[:, k * 512:(k + 1) * 512], start=True, stop=True)
                mm.then_inc(s_pe, 1)

        nc.compile()
    return nc


_NC_CACHE: list = []

# Optional profiling knobs (used by test harnesses; harmless defaults).
TRACE = False
TMPDIR = None
LAST_RESULT: list = []


def _get_nc():
    if not _NC_CACHE:
        _NC_CACHE.append(_build_nc())
    return _NC_CACHE[0]


def _host_prep_idx(cat_idx: np.ndarray) -> list[np.ndarray]:
    lin = cat_idx.astype(np.int64) + (np.arange(F, dtype=np.int64) * V)[None, :]
    lin = lin.astype(np.int32)                  # [B, F], values < 32032
    per_core = []
    for c in range(NCORES):
        sh = lin[c * BC:(c + 1) * BC].reshape(NT, P, F)
        per_core.append(np.ascontiguousarray(
            sh.transpose(1, 0, 2).reshape(P, NT * F)))
    return per_core


def _host_prep_num(numerical: np.ndarray) -> list[np.ndarray]:
    out = []
    for c in range(NCORES):
        sh = numerical[c * BC:(c + 1) * BC].reshape(NT, P, N_NUM)
        out.append(np.ascontiguousarray(
            sh.transpose(1, 0, 2).reshape(P, NT * N_NUM)))
    return out


def kernel(cat_idx, numerical, tables, gamma, beta):
    cat_idx = np.asarray(cat_idx)
    numerical = np.asarray(numerical, dtype=np.float32)
    tables = np.asarray(tables, dtype=np.float32)
    gamma = np.asarray(gamma, dtype=np.float32)
    beta = np.asarray(beta, dtype=np.float32)

    nc = _get_nc()
    tab_flat = np.ascontiguousarray(tables.reshape(R, D))
    gb = np.ascontiguousarray(
        np.stack([gamma.reshape(FD), beta.reshape(FD)], axis=0))
    idx_pc = _host_prep_idx(cat_idx)
    num_pc = _host_prep_num(numerical)

    in_maps = [
        {"tab": tab_flat, "idx": idx_pc[c], "nume": num_pc[c], "gb": gb}
        for c in range(NCORES)
    ]
    res = run_bass_kernel_spmd(nc, in_maps, core_ids=list(range(NCORES)),
                               trace=TRACE, tmpdir=TMPDIR)
    LAST_RESULT.clear()
    LAST_RESULT.append(res)
    out = np.concatenate([res.results[c]["out"] for c in range(NCORES)], axis=0)
    return out

